# revision 5
# baseline (speedup 1.0000x reference)
"""Trainium2 Bass kernel v3: partition-split layout.

Feature dim H=256 is split across partition halves: a state tensor is
[128, 128] where partition p<64 holds (batch b=p, features 0:128) and
p>=64 holds (batch b=p-64, features 128:256).

Per step:
  - gates PSUM bank [128, 4*128] with slots [g, i, f, o];
    written by col-tiled matmul pairs: tile_position (0,0) computes the
    feature-half-0 gates into partitions 0:64, (0,64) computes half-1
    into partitions 64:128 (concurrently on the PE array).
  - ONE sigmoid over [i,f,o] = ps[:,128:512], ONE tanh over g.
  - DVE: fc, ig, cadd (each [128,128] = half the free-dim of v2).
  - tail: transpose c2 -> tanh^T on ACT; transpose o; hT = o^T * tanh^T
    (one DVE op writes the next step's stationary operand directly).
All matmul operands bf16; x resident in SBUF; y accumulated in SBUF.
"""

import numpy as np
from contextlib import ExitStack

import concourse.bass as bass
import concourse.bacc as bacc
import concourse.mybir as mybir
import concourse.tile as tile
from concourse.tile import add_dep_helper
from concourse.bass_utils import run_bass_kernel_spmd

B, T, D, H = 512, 512, 64, 256
G4 = 4 * H
NCORES = 8
BL = B // NCORES  # 64
TDEC = T - 1
BIG = 30000.0
F32 = mybir.dt.float32
F32R = mybir.dt.float32r
BF16 = mybir.dt.bfloat16

_PROGRAM = None
_LAST_RES = None

# slot order in the gate bank
SLOT_GATE = [2, 0, 1, 3]  # slots [g, i, f, o] -> torch gate idx (i=0,f=1,g=2,o=3)


def build_program(t_enc=T, t_dec=TDEC, col_tiled=True):
    nc = bacc.Bacc(None, target_bir_lowering=False)
    f = F32
    xp_d = nc.dram_tensor("xp", [66, t_enc, BL], BF16, kind="ExternalInput")
    x0p_d = nc.dram_tensor("x0p", [65, BL], BF16, kind="ExternalInput")
    wxenc_d = nc.dram_tensor("wxenc", [66, G4], BF16, kind="ExternalInput")
    whhenc_d = nc.dram_tensor("whhenc", [128, 2, G4], BF16, kind="ExternalInput")
    whhdec_d = nc.dram_tensor("whhdec", [128, 2, G4], BF16, kind="ExternalInput")
    wcomb_d = nc.dram_tensor("wcomb", [128, 2, G4], BF16, kind="ExternalInput")
    wxdec_d = nc.dram_tensor("wxdec", [65, G4], BF16, kind="ExternalInput")
    bcomb_d = nc.dram_tensor("bcomb", [1, G4], BF16, kind="ExternalInput")
    outw_d = nc.dram_tensor("outw", [128, 2, D], BF16, kind="ExternalInput")
    outbc_d = nc.dram_tensor("outbc", [D, 1], f, kind="ExternalInput")
    masks_d = nc.dram_tensor("masks", [128, t_enc], f, kind="ExternalInput")
    ident_d = nc.dram_tensor("ident", [128, 128], f, kind="ExternalInput")
    yt_d = nc.dram_tensor("yt", [D, t_dec + 1, BL], BF16, kind="ExternalOutput")

    Sig = mybir.ActivationFunctionType.Sigmoid
    Tanh = mybir.ActivationFunctionType.Tanh

    with ExitStack() as ctx:
        tc = ctx.enter_context(tile.TileContext(nc))
        singles = ctx.enter_context(tc.tile_pool(name="singles", bufs=1))
        work = ctx.enter_context(tc.tile_pool(name="work", bufs=3))
        cpool = ctx.enter_context(tc.tile_pool(name="cpool", bufs=2))
        htp = ctx.enter_context(tc.tile_pool(name="htp", bufs=2))
        oap = ctx.enter_context(tc.tile_pool(name="oap", bufs=2))
        gpool = ctx.enter_context(
            tc.tile_pool(name="gpool", bufs=2, space=bass.MemorySpace.PSUM))
        tcp = ctx.enter_context(
            tc.tile_pool(name="tcp", bufs=2, space=bass.MemorySpace.PSUM))
        top = ctx.enter_context(
            tc.tile_pool(name="top", bufs=2, space=bass.MemorySpace.PSUM))
        ypool = ctx.enter_context(
            tc.tile_pool(name="ypool", bufs=2, space=bass.MemorySpace.PSUM))

        # ---- persistent constants / resident data ----
        s_xp = singles.tile([66, t_enc, BL], BF16)
        nc.sync.dma_start(s_xp, xp_d[:, :, :])
        s_wxenc = singles.tile([66, G4], BF16)
        nc.sync.dma_start(s_wxenc, wxenc_d[:, :])
        s_whhenc = singles.tile([128, 2, G4], BF16)
        nc.sync.dma_start(s_whhenc, whhenc_d[:, :, :])
        s_whhdec = singles.tile([128, 2, G4], BF16)
        nc.sync.dma_start(s_whhdec, whhdec_d[:, :, :])
        s_wcomb = singles.tile([128, 2, G4], BF16)
        nc.sync.dma_start(s_wcomb, wcomb_d[:, :, :])
        s_wxdec = singles.tile([65, G4], BF16)
        nc.sync.dma_start(s_wxdec, wxdec_d[:, :])
        s_bcomb = singles.tile([1, G4], BF16)
        nc.sync.dma_start(s_bcomb, bcomb_d[:, :])
        s_outw = singles.tile([128, 2, D], BF16)
        nc.sync.dma_start(s_outw, outw_d[:, :, :])
        s_masks = singles.tile([128, t_enc], f)
        nc.sync.dma_start(s_masks, masks_d[:, :])
        s_ident0 = singles.tile([128, 128], f)
        nc.sync.dma_start(s_ident0, ident_d[:, :])
        s_x0p0 = singles.tile([65, BL], BF16)
        nc.sync.dma_start(s_x0p0, x0p_d[:, :])
        s_outbc = singles.tile([D, 1], f)
        nc.sync.dma_start(s_outbc, outbc_d[:, :])

        s_yacc = singles.tile([D, t_dec + 1, BL], BF16, tag="yacc")
        nc.vector.memset(s_yacc[:, 0, :], 0.0)

        s_ones0 = singles.tile([1, BL], f)
        nc.vector.memset(s_ones0, 1.0)
        s_ones = singles.tile([1, BL], BF16, tag="onesr")
        nc.vector.tensor_copy(s_ones, s_ones0)
        # first-touch of matmul operands through DVE (one sem)
        s_identf = singles.tile([128, 128], f, tag="identf")
        nc.vector.tensor_copy(s_identf, s_ident0)
        s_identb = singles.tile([128, 128], BF16, tag="identb")
        nc.vector.tensor_copy(s_identb, s_ident0)
        s_x0p = singles.tile([65, BL], BF16, tag="x0pv")
        nc.vector.tensor_copy(s_x0p, s_x0p0)
        s_bcomb0 = s_bcomb
        s_bcomb = singles.tile([1, G4], BF16, tag="bcombv")
        nc.vector.tensor_copy(s_bcomb, s_bcomb0)
        s_outw0 = s_outw
        s_outw = singles.tile([128, 2, D], BF16, tag="outwv")
        nc.vector.tensor_copy(s_outw, s_outw0)

        # ---- initial state (split layout [128, 128]) ----
        c_prev = singles.tile([128, 128], f, tag="c0")
        nc.vector.memset(c_prev, 0.0)
        hT0f = singles.tile([128, 128], f, tag="ht0f")
        nc.vector.memset(hT0f, 0.0)
        hT_prev = singles.tile([128, 128], BF16, tag="ht0")
        nc.vector.tensor_copy(hT_prev, hT0f)
        o_acc = singles.tile([128, 128], f, tag="oacc0")
        nc.vector.memset(o_acc, 0.0)

        def chain_order(*insts):
            for a, b in zip(insts[1:], insts[:-1]):
                add_dep_helper(a.ins, b.ins, sync=False, reason="order")

        def open_banks(lhs, rhs):
            """New gate bank; write x/bias part for both partition halves."""
            ps = gpool.tile([128, 512], f, tag="g")
            if col_tiled:
                nc.tensor.matmul(ps[0:64, :], lhs, rhs[:, 0:512],
                                 start=True, stop=False, tile_position=(0, 0),
                                 skip_group_check=True)
                nc.tensor.matmul(ps[64:128, :], lhs, rhs[:, 512:1024],
                                 start=True, stop=False, tile_position=(0, 64),
                                 skip_group_check=True)
            else:
                nc.tensor.matmul(ps[0:64, :], lhs, rhs[:, 0:512],
                                 start=True, stop=False,
                                 skip_group_check=True)
                nc.tensor.matmul(ps[64:128, :], lhs, rhs[:, 512:1024],
                                 start=True, stop=False,
                                 skip_group_check=True)
            return ps

        def h_matmuls(ps, whh):
            last = None
            for k in range(2):
                lhsT = hT_prev[:, 64 * k:64 * k + 64]
                stop = (k == 1)
                if col_tiled:
                    ma = nc.tensor.matmul(ps[0:64, :], lhsT, whh[:, k, 0:512],
                                          start=False, stop=stop,
                                          tile_position=(0, 0),
                                          skip_group_check=True)
                    mb = nc.tensor.matmul(ps[64:128, :], lhsT,
                                          whh[:, k, 512:1024],
                                          start=False, stop=stop,
                                          tile_position=(0, 64),
                                          skip_group_check=True)
                else:
                    ma = nc.tensor.matmul(ps[0:64, :], lhsT, whh[:, k, 0:512],
                                          start=False, stop=stop,
                                          skip_group_check=True)
                    mb = nc.tensor.matmul(ps[64:128, :], lhsT,
                                          whh[:, k, 512:1024],
                                          start=False, stop=stop,
                                          skip_group_check=True)
                last = mb
            return last

        def cell(ps, masked_t):
            """gates -> new c, hT; returns last DVE inst of the tail."""
            nonlocal c_prev, o_acc, hT_prev
            s_ifo = work.tile([128, 384], BF16, tag="sifo")
            a1 = nc.scalar.activation(s_ifo, ps[:, 128:512], Sig)
            g_t = work.tile([128, 128], BF16, tag="gt")
            a2 = nc.scalar.activation(g_t, ps[:, 0:128], Tanh)
            chain_order(a1, a2)
            i_t = s_ifo[:, 0:128]
            f_t = s_ifo[:, 128:256]
            o_t = s_ifo[:, 256:384]
            fc = work.tile([128, 128], f, tag="fc")
            d1 = nc.vector.tensor_mul(fc, f_t, c_prev)
            ig = work.tile([128, 128], BF16, tag="ig")
            d2 = nc.vector.tensor_mul(ig, i_t, g_t)
            c_new = cpool.tile([128, 128], f, tag="c")
            d3 = nc.vector.tensor_add(c_new, fc, ig)
            chain_order(d1, d2, d3)
            # tail: transpose c2, tanh on ACT, transpose o, fuse h-mul+copy
            tpc = tcp.tile([128, 128], f, tag="tpc")
            nc.tensor.transpose(tpc, c_new, s_identf)
            tcT = work.tile([128, 128], BF16, tag="tcT")
            nc.scalar.activation(tcT, tpc, Tanh)
            tpo = top.tile([128, 128], BF16, tag="tpo")
            nc.tensor.transpose(tpo, o_t, s_identb)
            hT = htp.tile([128, 128], BF16, tag="hT")
            hm = nc.vector.tensor_mul(hT, tpo, tcT)
            chain_order(d3, hm)
            if masked_t is not None:
                oam = work.tile([128, 128], f, tag="oam")
                om = nc.gpsimd.tensor_scalar_mul(
                    oam, o_t, s_masks[:, masked_t:masked_t + 1])
                o_acc2 = oap.tile([128, 128], f, tag="oacc")
                oa = nc.gpsimd.tensor_add(o_acc2, o_acc, oam)
                chain_order(om, oa)
                o_acc = o_acc2
            c_prev = c_new
            hT_prev = hT
            return hm, d3

        # ================= ENCODER =================
        ps = open_banks(s_xp[:, 0, :], s_wxenc)
        for t in range(t_enc):
            h_matmuls(ps, s_whhenc)
            cell(ps, t)
            if t + 1 < t_enc:
                ps = open_banks(s_xp[:, t + 1, :], s_wxenc)

        # ===== boundary: h_enc = o_acc * tanh(c_final) =====
        ps = open_banks(s_x0p, s_wxdec)
        tc_e = work.tile([128, 128], f, tag="tce")
        nc.scalar.activation(tc_e, c_prev, Tanh)
        h_enc = work.tile([128, 128], f, tag="henc")
        nc.vector.tensor_mul(h_enc, o_acc, tc_e)
        tpb = tcp.tile([128, 128], f, tag="tpc")
        nc.tensor.transpose(tpb, h_enc, s_identf)
        hTb = htp.tile([128, 128], BF16, tag="hT")
        nc.vector.tensor_copy(hTb, tpb)
        hT_prev = hTb

        # ================= DECODER =================
        pending_y = None

        def emit_y(last_h, after_c):
            hTp, slot = pending_y
            yps = ypool.tile([D, BL], f, tag="y")
            for kc in range(2):
                ym = nc.tensor.matmul(yps, s_outw[:, kc, :],
                                      hTp[:, 64 * kc:64 * kc + 64],
                                      start=(kc == 0), stop=(kc == 1))
                if last_h is not None:
                    add_dep_helper(ym.ins, last_h.ins, sync=False,
                                   reason="y after h MMs")
            ya = nc.vector.tensor_scalar_add(s_yacc[:, slot, :], yps, s_outbc)
            if after_c is not None:
                add_dep_helper(ya.ins, after_c.ins, sync=False,
                               reason="y after c2")
            return ya

        for j in range(t_dec):
            whh = s_whhdec if j == 0 else s_wcomb
            last_h = h_matmuls(ps, whh)
            hm, d3 = cell(ps, None)
            if pending_y is not None:
                emit_y(last_h, d3)
            if j + 1 < t_dec:
                ps = open_banks(s_ones, s_bcomb)
            pending_y = (hT_prev, j + 1)
        emit_y(None, None)

        nc.sync.dma_start(yt_d[:, :, :], s_yacc)

    nc.compile()
    return nc


def _col_perm(hh):
    """Column indices into [4H]-gate space for slot order [g,i,f,o], half hh."""
    cols = []
    for s in range(4):
        g = SLOT_GATE[s]
        cols.append(np.arange(128) + g * H + hh * 128)
    return np.concatenate(cols)


def _prep_host(inputs, t_enc=T, t_dec=TDEC):
    import ml_dtypes
    bf16 = ml_dtypes.bfloat16
    x = np.asarray(inputs["input_tensor"], np.float32)
    tgt = np.asarray(inputs["target_tensor"], np.float32)
    lens = np.asarray(inputs["lens"]).astype(np.int64)

    eWih = np.asarray(inputs["enc_Wih"], np.float32)
    eWhh = np.asarray(inputs["enc_Whh"], np.float32)
    eb = (np.asarray(inputs["enc_bih"], np.float32)
          + np.asarray(inputs["enc_bhh"], np.float32))
    dWih = np.asarray(inputs["dec_Wih"], np.float32)
    dWhh = np.asarray(inputs["dec_Whh"], np.float32)
    db = (np.asarray(inputs["dec_bih"], np.float32)
          + np.asarray(inputs["dec_bhh"], np.float32))
    oW = np.asarray(inputs["out_W"], np.float32)
    ob = np.asarray(inputs["out_b"], np.float32)

    pA, pB = _col_perm(0), _col_perm(1)

    freeze = np.zeros(G4, np.float32)
    freeze[0:H] = -BIG      # i -> 0 (torch gate rows)
    freeze[H:2 * H] = BIG   # f -> 1

    def arrange_x(Wih, bvec, with_freeze):
        # rows: [Wih^T (K_in), bias, freeze?] cols arranged [A(512) B(512)]
        k_in = Wih.shape[1]
        nrows = k_in + 1 + (1 if with_freeze else 0)
        out = np.zeros((nrows, G4), np.float32)
        full = np.concatenate([Wih.T, bvec[None, :]], 0)  # [k+1, 4H]
        if with_freeze:
            full = np.concatenate([full, freeze[None, :]], 0)
        out[:, 0:512] = full[:, pA]
        out[:, 512:1024] = full[:, pB]
        return out

    def arrange_h(Whh):
        # [128, 2, 1024]: [kappa, k, A/B cols]
        Wt = Whh.T  # [H, 4H]
        out = np.zeros((128, 2, G4), np.float32)
        for k in range(2):
            blk = Wt[128 * k:128 * (k + 1)]  # [128, 4H]
            out[:, k, 0:512] = blk[:, pA]
            out[:, k, 512:1024] = blk[:, pB]
        return out

    wxenc = arrange_x(eWih, eb, True)              # [66, 1024]
    wxdec = arrange_x(dWih, db, False)             # [65, 1024]
    whhencT = arrange_h(eWhh)
    whhdecT = arrange_h(dWhh)
    wcomb = dWhh + dWih @ oW
    wcombT = arrange_h(wcomb)
    bcombv = db + dWih @ ob
    bcomb = np.zeros((1, G4), np.float32)
    bcomb[0, 0:512] = bcombv[pA]
    bcomb[0, 512:1024] = bcombv[pB]
    outwT = oW.T.reshape(2, 128, D).transpose(1, 0, 2).copy()
    ident = np.eye(128, dtype=np.float32)

    tt = np.arange(t_enc)[None, :]
    in_maps = []
    for c in range(NCORES):
        b0 = c * BL
        xs = x[b0:b0 + BL, :t_enc, :]
        xp = np.empty((66, t_enc, BL), np.float32)
        xp[0:D] = xs.transpose(2, 1, 0)
        xp[D] = 1.0
        lc = lens[b0:b0 + BL]
        mbar = (tt >= lc[:, None]).astype(np.float32)   # [BL, t]
        xp[D + 1] = mbar.T
        efreeze = (tt == (lc[:, None] - 1)).astype(np.float32)  # [BL,t]
        masks = np.concatenate([efreeze, efreeze], 0)   # [128, t] dup halves
        x0p = np.empty((65, BL), np.float32)
        x0p[0:D, :] = tgt[b0:b0 + BL, 0, :].T
        x0p[D, :] = 1.0
        in_maps.append({
            "xp": np.ascontiguousarray(xp).astype(bf16),
            "x0p": x0p.astype(bf16),
            "wxenc": wxenc.astype(bf16), "whhenc": whhencT.astype(bf16),
            "whhdec": whhdecT.astype(bf16), "wcomb": wcombT.astype(bf16),
            "wxdec": wxdec.astype(bf16), "bcomb": bcomb.astype(bf16),
            "outw": outwT.astype(bf16), "outbc": ob[:, None].copy(),
            "masks": np.ascontiguousarray(masks),
            "ident": ident,
        })
    return in_maps, lens


def kernel(**inputs) -> np.ndarray:
    global _PROGRAM, _LAST_RES
    if _PROGRAM is None:
        _PROGRAM = build_program()
    nc = _PROGRAM
    in_maps, lens = _prep_host(inputs)
    import time as _time
    _t0 = _time.time()
    res = run_bass_kernel_spmd(nc, in_maps, core_ids=list(range(NCORES)))
    _LAST_RES = res
    _LAST_RES.run_wall_s = _time.time() - _t0
    out = np.zeros((B, T, D), np.float32)
    for c in range(NCORES):
        yt = res.results[c]["yt"].astype(np.float32)   # [D, T, BL]
        out[c * BL:(c + 1) * BL] = yt.transpose(2, 1, 0)
    mask = (np.arange(T)[None, :] < lens[:, None])[:, :, None]
    out *= mask
    out[:, 0, :] = 0.0
    return out
